# revision 1
# baseline (speedup 1.0000x reference)
"""Trainium2 Bass kernel for CausalCrossConditionalSelfAttention.

Reference semantics (B=2, T=2560, C=768, H=12, hd=64, t=T//10=256):
  q/k/v = x @ W{q,k,v}.T + b{q,k,v}           (per-head slices of C)
  att   = softmax(mask(q k^T / sqrt(hd)))      mask: (i%256) >= (j%256)
  y     = (att @ v) @ Wp.T + bp

Sharding: 8 cores = 2 batches x 4 head-groups (3 heads each).
Each core computes its (batch, 3 heads) slab fully on-chip and returns a
partial pre-projection output out^T [768, 2560]; the host sums the 4
head-group partials per batch and adds the constant bias (bp + Wp @ bv).

Device-side layout (per core):
  xT      [768, 2560]  x[b].T
  wqk     [768, 384]   cols: [Qh0|Qh1 | Kh0|Kh1 | Qh2 | Kh2] weight.T cols
  bqk     [4, 128, 1]  per-partition bias rows for the 4 col-groups
  wv      [768, 192]   Wv rows for the 3 heads, transposed
  wp      [3, 64, 768] per-head Wp[:, head_slice].T
  out     [768, 2560]  partial out^T (pre-bias)

The scores are computed transposed: S^T[k, q] in PSUM, exp'd on ScalarE
(scale=1/8 fused), masked by GPSIMD affine_select (exact zeros), and
contracted with V (ones column appended -> softmax denominator for free).
The (k%256)>=128 x (q%256)<128 quarter of each 256x256 mask block is fully
masked and skipped entirely (25% of score/AV/exp work).
"""

import numpy as np

B, T, C = 2, 2560, 768
H, HD = 12, 64
HPG = 3            # heads per group (core)
CW = HPG * HD      # 192
NKC = T // 128     # 20 key chunks of 128
NQT = T // 512     # 5 query tiles of 512
N_CORES = 8

_CACHE = {}


def _split_multi_waits(nc, maxw=1):
    """walrus in this container rejects >1 sync wait per instruction;
    split extra waits onto preceding NOPs on the same engine."""
    import concourse.mybir as mybir
    for f in nc.m.functions:
        for bb in f.blocks:
            newlist = []
            for ins in bb.instructions:
                si = ins.sync_info
                if si is not None and si.on_wait and len(si.on_wait) > maxw:
                    waits = list(si.on_wait)
                    chunks = [waits[i:i + maxw] for i in range(0, len(waits), maxw)]
                    for ch in chunks[:-1]:
                        newlist.append(mybir.InstNoOp(
                            name=f"WSPLIT-{nc.next_id()}",
                            engine=ins.engine,
                            sync_info=mybir.SyncInfo(on_wait=list(ch), on_update=[]),
                            text_hint="wait_split",
                        ))
                    ins.sync_info = mybir.SyncInfo(
                        on_wait=list(chunks[-1]), on_update=list(si.on_update))
                newlist.append(ins)
            bb.instructions = newlist
    return nc


def _chunks(lst, n):
    return [lst[i:i + n] for i in range(0, len(lst), n)]


def build_program():
    import concourse.bass as bass
    import concourse.mybir as mybir
    import concourse.tile as tile

    f32 = mybir.dt.float32
    bf16 = mybir.dt.bfloat16
    AF = mybir.ActivationFunctionType
    ALU = mybir.AluOpType

    nc = bass.Bass()
    xT = nc.dram_tensor("xT", [C, T], bf16, kind="ExternalInput")
    wqk = nc.dram_tensor("wqk", [C, 384], bf16, kind="ExternalInput")
    bqk = nc.dram_tensor("bqk", [4, 128, 1], f32, kind="ExternalInput")
    wv = nc.dram_tensor("wv", [C, CW], bf16, kind="ExternalInput")
    wp = nc.dram_tensor("wp", [HPG, HD, C], bf16, kind="ExternalInput")
    identm = nc.dram_tensor("identm", [128, 384], f32, kind="ExternalInput")
    out = nc.dram_tensor("out", [C, T], f32, kind="ExternalOutput")
    rcpb = nc.dram_tensor("rcpb", [HPG * NQT, 512], f32)

    with tile.TileContext(nc) as tc:
        with tc.tile_pool(name="persist", bufs=1) as persist, \
             tc.tile_pool(name="work", bufs=2) as work, \
             tc.tile_pool(name="psum", bufs=2, space="PSUM") as psum:

            # ---------------- load inputs ----------------
            wqk_sb = persist.tile([128, 6, 384], bf16)
            for c in range(6):
                nc.sync.dma_start(out=wqk_sb[:, c, :],
                                  in_=wqk[c * 128:(c + 1) * 128, :])
            wv_sb = persist.tile([128, 6, CW], bf16)
            for c in range(6):
                nc.sync.dma_start(out=wv_sb[:, c, :],
                                  in_=wv[c * 128:(c + 1) * 128, :])
            wp_sb = persist.tile([64, HPG, C], bf16)
            for h in range(HPG):
                nc.sync.dma_start(out=wp_sb[:, h, :], in_=wp[h])
            bqk_sb = persist.tile([128, 4, 1], f32)
            for j in range(4):
                nc.sync.dma_start(out=bqk_sb[:, j, :], in_=bqk[j])
            xt_sb = persist.tile([128, 6, T], bf16)       # x^T, 6 chunks of C
            for qt in range(NQT):
                for c in range(6):
                    nc.sync.dma_start(
                        out=xt_sb[:, c, qt * 512:(qt + 1) * 512],
                        in_=xT[c * 128:(c + 1) * 128, qt * 512:(qt + 1) * 512])

            ones_sb = persist.tile([128, 64], f32)
            nc.vector.memset(ones_sb, 1.0)
            identm_f = work.tile([128, 384], f32, tag="im", bufs=1, name="identm_f")
            nc.sync.dma_start(out=identm_f, in_=identm[:, :])
            ident_sb = persist.tile([128, 128], bf16)   # identity
            maskm_sb = persist.tile([128, 256], bf16)   # [L|L], L=-1e9 if j<i
            nc.vector.tensor_copy(ident_sb, identm_f[:, 0:128])
            nc.vector.tensor_copy(maskm_sb, identm_f[:, 128:384])

            # ---------------- q/k projections (transposed) ----------------
            # qkT j0=[Qh0|Qh1] j1=[Kh0|Kh1] (128 parts); j2=Qh2 j3=Kh2 (64)
            qkT01q = persist.tile([128, T], bf16)
            qkT01k = persist.tile([128, T], bf16)
            qkT2q = persist.tile([64, T], bf16)
            qkT2k = persist.tile([64, T], bf16)
            jdefs = [(qkT01q, 0, 128), (qkT01k, 128, 128),
                     (qkT2q, 256, 64), (qkT2k, 320, 64)]
            for qt in range(NQT):
                for j, (dst, col0, m) in enumerate(jdefs):
                    qk_ps = psum.tile([128, 512], f32, tag="av", name="qk_ps")
                    for c in range(6):
                        nc.tensor.matmul(
                            qk_ps[:m, :],
                            lhsT=wqk_sb[:, c, col0:col0 + m],
                            rhs=xt_sb[:, c, qt * 512:(qt + 1) * 512],
                            start=(c == 0), stop=(c == 5))
                    nc.vector.tensor_scalar_add(
                        dst[:m, qt * 512:(qt + 1) * 512],
                        qk_ps[:m, :], bqk_sb[:m, j, :])

            # ---------------- v projection (natural layout + ones col) ----
            # per head h: cols [65h .. 65h+63] = V_h, col 65h+64 = 1.0
            v_sb = persist.tile([128, NKC, HPG * 65], bf16)
            v_r = v_sb.rearrange("p n (h c) -> p n h c", c=65)
            nc.vector.memset(v_r[:, :, :, 64], 1.0)
            for tch in range(NKC):
                v_ps = psum.tile([128, 512], f32, tag="av", name="v_ps")
                for c in range(6):
                    nc.tensor.matmul(
                        v_ps[:, :CW],
                        lhsT=xt_sb[:, c, tch * 128:(tch + 1) * 128],
                        rhs=wv_sb[:, c, :],
                        start=(c == 0), stop=(c == 5))
                nc.vector.tensor_copy(
                    v_r[:, tch, :, 0:64],
                    v_ps[:, :CW].rearrange("p (h c) -> p h c", h=HPG))

            # ---------------- attention ----------------
            evens = list(range(0, NKC, 2))
            odds = list(range(1, NKC, 2))
            ynorm = [persist.tile([64, T], bf16, name=f"ynorm{h}")
                     for h in range(HPG)]

            pending = []

            def _emit_proj(qt_p):
                for m in range(6):
                    pj_ps = psum.tile([128, 512], f32, tag="sc", name="pj_ps")
                    for hh in range(HPG):
                        nc.tensor.matmul(
                            pj_ps,
                            lhsT=wp_sb[:, hh, m * 128:(m + 1) * 128],
                            rhs=ynorm[hh][:, qt_p * 512:(qt_p + 1) * 512],
                            start=(hh == 0), stop=(hh == 2))
                    pj_sb = work.tile([128, 512], f32, tag="pj", name="pj_sb")
                    nc.vector.tensor_copy(pj_sb, pj_ps)
                    nc.sync.dma_start(
                        out=out[m * 128:(m + 1) * 128,
                                qt_p * 512:(qt_p + 1) * 512],
                        in_=pj_sb)

            def _flush_norm(item):
                av_p, rcp_p, h_p, qt_p = item
                slot = h_p * NQT + qt_p
                bc_sb = work.tile([64, 512], f32, tag="bc", name="bc_sb")
                nc.sync.dma_start(out=rcpb[slot:slot+1, :], in_=rcp_p[64:65, :])
                bcast_in = bass.AP(tensor=rcpb, offset=slot * 512,
                                   ap=[[0, 64], [1, 512]])
                nc.sync.dma_start(out=bc_sb, in_=bcast_in)
                nc.vector.tensor_mul(
                    ynorm[h_p][:, qt_p * 512:(qt_p + 1) * 512],
                    av_p[0:64, :], bc_sb)
                if h_p == HPG - 1:
                    _emit_proj(qt_p)

            for qt in range(NQT):
                for h in range(HPG):
                    if h < 2:
                        qTh = qkT01q[64 * h:64 * (h + 1), :]
                        kTh = qkT01k[64 * h:64 * (h + 1), :]
                    else:
                        qTh = qkT2q[0:64, :]
                        kTh = qkT2k[0:64, :]
                    # odd-subchunk view of q: [64, qt, two, sp, 128]
                    q_odd = qTh.rearrange("p (q s t c) -> p q t s c",
                                          q=NQT, s=2, t=2, c=128)
                    qwin = qTh[:, qt * 512:(qt + 1) * 512]
                    av = psum.tile([128, 512], f32, tag="av", name="av")
                    av_odd = av.rearrange("p (s t c) -> p t s c",
                                          s=2, t=2, c=128)[:, 1]

                    for grp in _chunks(evens, 3):
                        L = len(grp)
                        sc = psum.tile([128, 1536], f32, tag="sc", name="sc")
                        for i, kc in enumerate(grp):
                            nc.tensor.matmul(
                                sc[:, i * 512:(i + 1) * 512],
                                lhsT=kTh[:, kc * 128:(kc + 1) * 128],
                                rhs=qwin, start=True, stop=False,
                                skip_group_check=True)
                        sc_r = sc.rearrange("p (l s t c) -> p l s t c",
                                            l=3, s=2, t=2, c=128)
                        for i in range(L):
                            nc.tensor.matmul(
                                sc_r[:, i, :, 0], lhsT=ident_sb, rhs=maskm_sb,
                                start=False, stop=True, skip_group_check=True)
                        pt = work.tile([128, 1536], bf16, tag="pt", name="pt")
                        nc.scalar.activation(pt[:, :L * 512], sc[:, :L * 512],
                                             AF.Exp, scale=0.125)
                        for i, kc in enumerate(grp):
                            nc.tensor.matmul(
                                av[:65, :],
                                lhsT=v_sb[:, kc, 65 * h:65 * h + 65],
                                rhs=pt[:, i * 512:(i + 1) * 512],
                                start=(kc == 0), stop=False,
                                skip_group_check=True)

                    if pending:
                        _flush_norm(pending.pop(0))
                    for gi, grp in enumerate(_chunks(odds, 3)):
                        L = len(grp)
                        last_grp = (gi == 3)
                        sc = psum.tile([128, 1536], f32, tag="sc", name="sc")
                        for i, kc in enumerate(grp):
                            # 256-wide blocks: two share a 2KB psum bank, and
                            # start=True zero-marks the WHOLE bank - only the
                            # first block of each bank may set it.
                            nc.tensor.matmul(
                                sc[:, i * 256:(i + 1) * 256],
                                lhsT=kTh[:, kc * 128:(kc + 1) * 128],
                                rhs=q_odd[:, qt, 1], start=(i % 2 == 0),
                                stop=False, skip_group_check=True)
                        for i in range(L):
                            nc.tensor.matmul(
                                sc[:, i * 256:(i + 1) * 256],
                                lhsT=ident_sb, rhs=maskm_sb,
                                start=False, stop=True, skip_group_check=True)
                        pt = work.tile([128, 1536], bf16, tag="pt", name="pt")
                        nc.scalar.activation(pt[:, :L * 256], sc[:, :L * 256],
                                             AF.Exp, scale=0.125)
                        for i, kc in enumerate(grp):
                            nc.tensor.matmul(
                                av_odd[:65],
                                lhsT=v_sb[:, kc, 65 * h:65 * h + 65],
                                rhs=pt[:, i * 256:(i + 1) * 256],
                                start=False, stop=(kc == NKC - 1),
                                skip_group_check=True)

                    # normalize: y = av[0:64] / av[64]  (denominator row)
                    rcp = work.tile([65, 512], f32, tag="rcp", name="rcp",
                                    bufs=3)
                    nc.vector.reciprocal(rcp[64:65, :], av[64:65, :])
                    pending.append((av, rcp, h, qt))

            while pending:
                _flush_norm(pending.pop(0))

    _split_multi_waits(nc)
    return nc


def get_program():
    if "nc" not in _CACHE:
        _CACHE["nc"] = build_program()
    return _CACHE["nc"]


def make_in_maps(x, Wk, bk, Wq, bq, Wv, bv, Wp, bp):
    x = np.asarray(x, dtype=np.float32)
    in_maps = []
    for core in range(N_CORES):
        b, g = divmod(core, 4)
        h0 = g * HPG
        r = slice(h0 * HD, (h0 + HPG) * HD)     # 192 head dims
        xt = np.ascontiguousarray(x[b].T)
        wq_g = np.asarray(Wq)[r]                 # [192, 768]
        wk_g = np.asarray(Wk)[r]
        # wqk cols: [Qh0|Qh1(128) | Kh0|Kh1(128) | Qh2(64) | Kh2(64)]
        wqk = np.concatenate(
            [wq_g[:128].T, wk_g[:128].T, wq_g[128:].T, wk_g[128:].T],
            axis=1).astype(np.float32)
        bq_g = np.asarray(bq)[r].astype(np.float32)
        bk_g = np.asarray(bk)[r].astype(np.float32)
        bqk = np.zeros((4, 128, 1), np.float32)
        bqk[0, :, 0] = bq_g[:128]
        bqk[1, :, 0] = bk_g[:128]
        bqk[2, :64, 0] = bq_g[128:]
        bqk[3, :64, 0] = bk_g[128:]
        wv_g = np.ascontiguousarray(np.asarray(Wv)[r].T).astype(np.float32)
        wp_g = np.asarray(Wp)[:, r]              # [768, 192]
        wp_t = np.ascontiguousarray(
            wp_g.T.reshape(HPG, HD, C)).astype(np.float32)
        ident = np.eye(128, dtype=np.float32)
        L = np.where(np.arange(256)[None, :] % 128 < np.arange(128)[:, None],
                     np.float32(-1e9), np.float32(0.0))
        identm = np.concatenate([ident, L], axis=1).astype(np.float32)
        import ml_dtypes
        b16 = ml_dtypes.bfloat16
        in_maps.append({
            "identm": identm,
            "xT": np.ascontiguousarray(xt).astype(b16),
            "wqk": np.ascontiguousarray(wqk).astype(b16),
            "bqk": bqk,
            "wv": wv_g.astype(b16),
            "wp": wp_t.astype(b16),
        })
    return in_maps


def kernel(x, Wk, bk, Wq, bq, Wv, bv, Wp, bp):
    from concourse.bass_utils import run_bass_kernel_spmd
    nc = get_program()
    in_maps = make_in_maps(x, Wk, bk, Wq, bq, Wv, bv, Wp, bp)
    res = run_bass_kernel_spmd(nc, in_maps, list(range(N_CORES)))
    Wp_np = np.asarray(Wp, dtype=np.float32)
    const = (np.asarray(bp, dtype=np.float32)
             + Wp_np @ np.asarray(bv, dtype=np.float32))   # [768]
    out = np.empty((B, T, C), dtype=np.float32)
    for b in range(B):
        acc = res.results[b * 4 + 0]["out"].astype(np.float32).copy()
        for g in range(1, 4):
            acc += res.results[b * 4 + g]["out"]
        out[b] = acc.T + const[None, :]
    return out



# revision 3
# speedup vs baseline: 1.0460x; 1.0460x over previous
"""Trainium2 Bass kernel for CausalCrossConditionalSelfAttention.

Reference semantics (B=2, T=2560, C=768, H=12, hd=64, t=T//10=256):
  q/k/v = x @ W{q,k,v}.T + b{q,k,v}           (per-head slices of C)
  att   = softmax(mask(q k^T / sqrt(hd)))      mask: (q%256) >= (k%256)
  y     = (att @ v) @ Wp.T + bp

Sharding: 8 cores = 2 batches x 4 head-groups (3 heads each).
Each core computes its (batch, 3 heads) slab fully on-chip and returns a
partial pre-projection output out^T [768, 2560]; the host sums the 4
head-group partials per batch and adds the constant bias (bp + Wp @ bv).

v2 structure (per core):
  - qk projections (4 col-groups x 5 q-windows), v projection interleaved
    per window; bias adds on VectorE.
  - attention per unit (qt, h): 20 key chunks split into 5 uniform groups
    of 1536 score columns each (3 even chunks x 512, or mixed 512+256-wide
    odd blocks).  Mask written FIRST into PSUM via ident x maskm matmul
    with start=True (whole-bank has_written clear), scores accumulate on
    top, ScalarE exp (scale=1/8) -> bf16 pt, then V(+ones col) @ pt.
  - software pipelining: group g's AV matmuls are emitted after group
    g+1's score matmuls so the PE never sits behind the exp; the pipeline
    crosses unit boundaries.
  - softmax denominators: av[64] rows of the 3 heads of a q-window are
    copied to SBUF, bounced via DRAM into a [128,12] tile, ONE cheap
    reciprocal, bounced back, broadcast-DMA'd per head, one tensor_mul
    per head.  All bounce DMAs ride the idle GpSimd queue.
  - output projection per q-window deferred ~2 units so its inputs are
    always ready; results staged in SBUF and written with 2 wide DMAs.
"""

import numpy as np

B, T, C = 2, 2560, 768
H, HD = 12, 64
HPG = 3            # heads per group (core)
CW = HPG * HD      # 192
NKC = T // 128     # 20 key chunks of 128
NQT = T // 512     # 5 query tiles of 512
N_CORES = 8

_CACHE = {}


def _split_multi_waits(nc, maxw=1):
    """walrus in this container rejects >1 sync wait per instruction;
    split extra waits onto preceding NOPs on the same engine."""
    import concourse.mybir as mybir
    for f in nc.m.functions:
        for bb in f.blocks:
            newlist = []
            for ins in bb.instructions:
                si = ins.sync_info
                if si is not None and si.on_wait and len(si.on_wait) > maxw:
                    waits = list(si.on_wait)
                    chunks = [waits[i:i + maxw] for i in range(0, len(waits), maxw)]
                    for ch in chunks[:-1]:
                        newlist.append(mybir.InstNoOp(
                            name=f"WSPLIT-{nc.next_id()}",
                            engine=ins.engine,
                            sync_info=mybir.SyncInfo(on_wait=list(ch), on_update=[]),
                            text_hint="wait_split",
                        ))
                    ins.sync_info = mybir.SyncInfo(
                        on_wait=list(chunks[-1]), on_update=list(si.on_update))
                newlist.append(ins)
            bb.instructions = newlist
    return nc


def _unit_groups():
    """5 groups of 1536 score columns: [(kc, width, sc_offset), ...]."""
    gs = []
    for g in range(3):                      # E0,E1,E2: even chunks, 512 wide
        gs.append([(6 * g + 2 * i, 512, 512 * i) for i in range(3)])
    g3 = [(18, 512, 0)] + [(2 * i + 1, 256, 512 + 256 * i) for i in range(4)]
    gs.append(g3)                           # chunk 18 + odds 1,3,5,7
    gs.append([(2 * i + 1, 256, 256 * (i - 4)) for i in range(4, 10)])
    return gs                               # odds 9..19


def build_program():
    import concourse.bass as bass
    import concourse.mybir as mybir
    import concourse.tile as tile

    f32 = mybir.dt.float32
    bf16 = mybir.dt.bfloat16
    AF = mybir.ActivationFunctionType

    nc = bass.Bass()
    xT = nc.dram_tensor("xT", [C, T], bf16, kind="ExternalInput")
    wqk = nc.dram_tensor("wqk", [C, 384], bf16, kind="ExternalInput")
    bqk = nc.dram_tensor("bqk", [4, 128, 1], f32, kind="ExternalInput")
    wv = nc.dram_tensor("wv", [C, CW], bf16, kind="ExternalInput")
    wp = nc.dram_tensor("wp", [HPG, HD, C], bf16, kind="ExternalInput")
    identm = nc.dram_tensor("identm", [128, 640], f32, kind="ExternalInput")
    out = nc.dram_tensor("out", [C, T], f32, kind="ExternalOutput")
    denb = nc.dram_tensor("denb", [NQT, 1536], f32)
    rcpb = nc.dram_tensor("rcpb", [NQT, 1536], f32)

    with tile.TileContext(nc) as tc:
        with tc.tile_pool(name="persist", bufs=1) as persist, \
             tc.tile_pool(name="work", bufs=2) as work, \
             tc.tile_pool(name="psum", bufs=2, space="PSUM") as psum:

            # ---------------- input DMAs (critical path first) -------------
            wqk_sb = persist.tile([128, 6, 384], bf16)
            nc.sync.dma_start(out=wqk_sb, in_=bass.AP(
                tensor=wqk, offset=0,
                ap=[[384, 128], [128 * 384, 6], [1, 384]]))
            bqk_sb = persist.tile([128, 4], f32)
            nc.sync.dma_start(out=bqk_sb, in_=bass.AP(
                tensor=bqk, offset=0, ap=[[1, 128], [128, 4]]))
            xt_sb = persist.tile([128, 6, T], bf16)       # x^T, 6 chunks of C
            for c in range(6):
                nc.sync.dma_start(out=xt_sb[:, c, 0:512],
                                  in_=xT[c * 128:(c + 1) * 128, 0:512])
            # off-critical loads ride the gpsimd queue
            wv_sb = persist.tile([128, 6, CW], bf16)
            nc.gpsimd.dma_start(out=wv_sb, in_=bass.AP(
                tensor=wv, offset=0,
                ap=[[CW, 128], [128 * CW, 6], [1, CW]]))
            identm_f = work.tile([128, 640], f32, tag="im", bufs=1,
                                 name="identm_f")
            nc.gpsimd.dma_start(out=identm_f, in_=identm[:, :])
            wp_sb = persist.tile([64, HPG, C], bf16)
            for h in range(HPG):
                nc.gpsimd.dma_start(out=wp_sb[:, h, :], in_=wp[h])
            # rest of x
            for c in range(6):
                nc.sync.dma_start(out=xt_sb[:, c, 512:1024],
                                  in_=xT[c * 128:(c + 1) * 128, 512:1024])
            for c in range(6):
                nc.sync.dma_start(out=xt_sb[:, c, 1024:T],
                                  in_=xT[c * 128:(c + 1) * 128, 1024:T])

            ident_sb = persist.tile([128, 128], bf16)   # identity
            maskm4 = persist.tile([128, 512], bf16)     # [L|L|L|L]
            nc.vector.tensor_copy(ident_sb, identm_f[:, 0:128])
            nc.vector.tensor_copy(maskm4, identm_f[:, 128:640])

            # preload the Exp activation table off the critical path
            dummy = work.tile([128, 4], f32, tag="dmy", bufs=1, name="dummy")
            nc.scalar.activation(dummy, bqk_sb, AF.Exp, scale=0.0)

            # ---------------- q/k/v projections ----------------
            # qkT j0=[Qh0|Qh1] j1=[Kh0|Kh1] (128 parts); j2=Qh2 j3=Kh2 (64)
            qkT01q = persist.tile([128, T], bf16)
            qkT01k = persist.tile([128, T], bf16)
            qkT2q = persist.tile([64, T], bf16)
            qkT2k = persist.tile([64, T], bf16)
            jdefs = [(qkT01q, 0, 128, 0), (qkT01k, 128, 128, 1),
                     (qkT2q, 256, 64, 2), (qkT2k, 320, 64, 3)]

            # v natural layout + ones col: per head h cols [65h..65h+63]=V_h,
            # col 65h+64 = 1.0 (softmax denominator for free)
            v_sb = persist.tile([128, NKC, HPG * 65], bf16)
            v_r = v_sb.rearrange("p n (h c) -> p n h c", c=65)
            nc.vector.memset(v_r[:, :, :, 64], 1.0)

            for qt in range(NQT):
                for (dst, col0, m, j) in jdefs:
                    qk_ps = psum.tile([128, 512], f32, tag="av", name="qk_ps")
                    for c in range(6):
                        nc.tensor.matmul(
                            qk_ps[:m, :],
                            lhsT=wqk_sb[:, c, col0:col0 + m],
                            rhs=xt_sb[:, c, qt * 512:(qt + 1) * 512],
                            start=(c == 0), stop=(c == 5))
                    nc.vector.tensor_scalar_add(
                        dst[:m, qt * 512:(qt + 1) * 512],
                        qk_ps[:m, :], bqk_sb[:m, j:j + 1])
                for tch in range(4 * qt, 4 * qt + 4):
                    v_ps = psum.tile([128, 512], f32, tag="av", name="v_ps")
                    for c in range(6):
                        nc.tensor.matmul(
                            v_ps[:, :CW],
                            lhsT=xt_sb[:, c, tch * 128:(tch + 1) * 128],
                            rhs=wv_sb[:, c, :],
                            start=(c == 0), stop=(c == 5))
                    nc.vector.tensor_copy(
                        v_r[:, tch, :, 0:64],
                        v_ps[:, :CW].rearrange("p (h c) -> p h c", h=HPG))

            # ---------------- attention ----------------
            groups_def = _unit_groups()
            units = [(qt, h) for qt in range(NQT) for h in range(HPG)]
            avsb_q = {}
            bcs_q = {}
            pend = [None]  # (chunks, pt, av, h, last_of_unit, qt)

            def emit_masks(sc, G):
                for (kc, w, o) in G:
                    if w == 512:   # even chunk: L at t=0 half of each 256
                        mdst = sc[:, o:o + 512].rearrange(
                            "p (s t c) -> p s t c", s=2, t=2, c=128)[:, :, 0]
                        nc.tensor.matmul(mdst, lhsT=ident_sb,
                                         rhs=maskm4[:, 0:256],
                                         start=True, stop=False,
                                         skip_group_check=True)
                # odd chunks come in bank-aligned pairs: one 512-wide L|L|L|L
                odd_offsets = sorted(o for (kc, w, o) in G if w == 256)
                for o in odd_offsets[::2]:
                    nc.tensor.matmul(sc[:, o:o + 512], lhsT=ident_sb,
                                     rhs=maskm4,
                                     start=True, stop=False,
                                     skip_group_check=True)

            def emit_av(item):
                chunks, pt_t, av_t, h_t, last, qt_t = item
                av_odd = av_t.rearrange("p (s t c) -> p t s c",
                                        s=2, t=2, c=128)[:, 1]
                for (kc, w, o) in chunks:
                    if w == 512:
                        nc.tensor.matmul(
                            av_t[:65, :],
                            lhsT=v_sb[:, kc, 65 * h_t:65 * h_t + 65],
                            rhs=pt_t[:, o:o + 512],
                            start=(kc == 0), stop=(kc == NKC - 1),
                            skip_group_check=True)
                    else:
                        nc.tensor.matmul(
                            av_odd[:65],
                            lhsT=v_sb[:, kc, 65 * h_t:65 * h_t + 65],
                            rhs=pt_t[:, o:o + 256],
                            start=False, stop=(kc == NKC - 1),
                            skip_group_check=True)
                if last:
                    avsb = avsb_q[qt_t]
                    nc.vector.tensor_copy(avsb[:, h_t, :], av_t[0:65, :])
                    if h_t == HPG - 1:
                        emit_den_chain(qt_t)

            def emit_den_chain(qt_t):
                avsb = avsb_q[qt_t]
                nc.gpsimd.dma_start(
                    out=bass.AP(tensor=denb, offset=qt_t * 1536,
                                ap=[[0, 1], [512, HPG], [1, 512]]),
                    in_=avsb[64:65, :, :])
                d128 = work.tile([128, 12], f32, tag="d128", bufs=2,
                                 name="d128")
                nc.gpsimd.dma_start(out=d128, in_=bass.AP(
                    tensor=denb, offset=qt_t * 1536, ap=[[1, 128], [128, 12]]))
                r128 = work.tile([128, 12], f32, tag="r128", bufs=2,
                                 name="r128")
                nc.vector.reciprocal(r128, d128)
                nc.gpsimd.dma_start(
                    out=bass.AP(tensor=rcpb, offset=qt_t * 1536,
                                ap=[[1, 128], [128, 12]]),
                    in_=r128)
                bcs = []
                for h2 in range(HPG):
                    bc = work.tile([64, 512], f32, tag="bc", bufs=6,
                                   name="bc")
                    nc.gpsimd.dma_start(out=bc, in_=bass.AP(
                        tensor=rcpb, offset=qt_t * 1536 + h2 * 512,
                        ap=[[0, 64], [1, 512]]))
                    bcs.append(bc)
                bcs_q[qt_t] = bcs

            def emit_flush(qt_t):
                avsb = avsb_q.pop(qt_t)
                bcs = bcs_q.pop(qt_t)
                ynw = work.tile([64, HPG, 512], bf16, tag="ynw", bufs=2,
                                name="ynw")
                for h2 in range(HPG):
                    nc.vector.tensor_mul(ynw[:, h2, :], avsb[0:64, h2, :],
                                         bcs[h2])
                pjbuf = work.tile([128, 6, 512], f32, tag="pjb", bufs=2,
                                  name="pjbuf")
                for m in range(6):
                    pj_ps = psum.tile([128, 512], f32, tag="sc", name="pj_ps")
                    for hh in range(HPG):
                        nc.tensor.matmul(
                            pj_ps,
                            lhsT=wp_sb[:, hh, m * 128:(m + 1) * 128],
                            rhs=ynw[:, hh, :],
                            start=(hh == 0), stop=(hh == 2))
                    nc.vector.tensor_copy(pjbuf[:, m, :], pj_ps)
                    if m == 2:
                        nc.gpsimd.dma_start(
                            out=bass.AP(tensor=out, offset=qt_t * 512,
                                        ap=[[T, 128], [128 * T, 3], [1, 512]]),
                            in_=pjbuf[:, 0:3, :])
                nc.gpsimd.dma_start(
                    out=bass.AP(tensor=out, offset=3 * 128 * T + qt_t * 512,
                                ap=[[T, 128], [128 * T, 3], [1, 512]]),
                    in_=pjbuf[:, 3:6, :])

            for (qt, h) in units:
                if h == 0:
                    avsb_q[qt] = work.tile([65, HPG, 512], f32, tag="avsb",
                                           bufs=2, name="avsb")
                if h == 2 and qt >= 1:
                    emit_flush(qt - 1)
                if h < 2:
                    qTh = qkT01q[64 * h:64 * (h + 1), :]
                    kTh = qkT01k[64 * h:64 * (h + 1), :]
                else:
                    qTh = qkT2q[0:64, :]
                    kTh = qkT2k[0:64, :]
                q_odd = qTh.rearrange("p (q s t c) -> p q t s c",
                                      q=NQT, s=2, t=2, c=128)
                qwin = qTh[:, qt * 512:(qt + 1) * 512]
                av_t = psum.tile([128, 512], f32, tag="av", name="av")

                for gi, G in enumerate(groups_def):
                    sc = psum.tile([128, 1536], f32, tag="sc", name="sc")
                    emit_masks(sc, G)
                    for (kc, w, o) in G:
                        if w == 512:
                            nc.tensor.matmul(
                                sc[:, o:o + 512],
                                lhsT=kTh[:, kc * 128:(kc + 1) * 128],
                                rhs=qwin, start=False, stop=True,
                                skip_group_check=True)
                        else:
                            nc.tensor.matmul(
                                sc[:, o:o + 256],
                                lhsT=kTh[:, kc * 128:(kc + 1) * 128],
                                rhs=q_odd[:, qt, 1], start=False, stop=True,
                                skip_group_check=True)
                    if pend[0] is not None:
                        emit_av(pend[0])
                    pt_t = work.tile([128, 1536], bf16, tag="pt", bufs=3,
                                     name="pt")
                    nc.scalar.activation(pt_t, sc, AF.Exp, scale=0.125)
                    pend[0] = (G, pt_t, av_t, h,
                               gi == len(groups_def) - 1, qt)

            emit_av(pend[0])
            emit_flush(NQT - 1)

    _split_multi_waits(nc)
    return nc


def get_program():
    if "nc" not in _CACHE:
        _CACHE["nc"] = build_program()
    return _CACHE["nc"]


def make_in_maps(x, Wk, bk, Wq, bq, Wv, bv, Wp, bp):
    x = np.asarray(x, dtype=np.float32)
    in_maps = []
    for core in range(N_CORES):
        b, g = divmod(core, 4)
        h0 = g * HPG
        r = slice(h0 * HD, (h0 + HPG) * HD)     # 192 head dims
        xt = np.ascontiguousarray(x[b].T)
        wq_g = np.asarray(Wq)[r]                 # [192, 768]
        wk_g = np.asarray(Wk)[r]
        # wqk cols: [Qh0|Qh1(128) | Kh0|Kh1(128) | Qh2(64) | Kh2(64)]
        wqk = np.concatenate(
            [wq_g[:128].T, wk_g[:128].T, wq_g[128:].T, wk_g[128:].T],
            axis=1).astype(np.float32)
        bq_g = np.asarray(bq)[r].astype(np.float32)
        bk_g = np.asarray(bk)[r].astype(np.float32)
        bqk = np.zeros((4, 128, 1), np.float32)
        bqk[0, :, 0] = bq_g[:128]
        bqk[1, :, 0] = bk_g[:128]
        bqk[2, :64, 0] = bq_g[128:]
        bqk[3, :64, 0] = bk_g[128:]
        wv_g = np.ascontiguousarray(np.asarray(Wv)[r].T).astype(np.float32)
        wp_g = np.asarray(Wp)[:, r]              # [768, 192]
        wp_t = np.ascontiguousarray(
            wp_g.T.reshape(HPG, HD, C)).astype(np.float32)
        ident = np.eye(128, dtype=np.float32)
        L = np.where(np.arange(128)[None, :] < np.arange(128)[:, None],
                     np.float32(-1e9), np.float32(0.0))
        identm = np.concatenate([ident, np.tile(L, (1, 4))],
                                axis=1).astype(np.float32)
        import ml_dtypes
        b16 = ml_dtypes.bfloat16
        in_maps.append({
            "identm": identm,
            "xT": np.ascontiguousarray(xt).astype(b16),
            "wqk": np.ascontiguousarray(wqk).astype(b16),
            "bqk": bqk,
            "wv": wv_g.astype(b16),
            "wp": wp_t.astype(b16),
        })
    return in_maps


def kernel(x, Wk, bk, Wq, bq, Wv, bv, Wp, bp):
    from concourse.bass_utils import run_bass_kernel_spmd
    nc = get_program()
    in_maps = make_in_maps(x, Wk, bk, Wq, bq, Wv, bv, Wp, bp)
    res = run_bass_kernel_spmd(nc, in_maps, list(range(N_CORES)))
    Wp_np = np.asarray(Wp, dtype=np.float32)
    const = (np.asarray(bp, dtype=np.float32)
             + Wp_np @ np.asarray(bv, dtype=np.float32))   # [768]
    out = np.empty((B, T, C), dtype=np.float32)
    for b in range(B):
        acc = res.results[b * 4 + 0]["out"].astype(np.float32).copy()
        for g in range(1, 4):
            acc += res.results[b * 4 + g]["out"]
        out[b] = acc.T + const[None, :]
    return out


# revision 11
# speedup vs baseline: 1.0705x; 1.0235x over previous
"""Trainium2 Bass kernel for CausalCrossConditionalSelfAttention.

Reference semantics (B=2, T=2560, C=768, H=12, hd=64, t=T//10=256):
  q/k/v = x @ W{q,k,v}.T + b{q,k,v}           (per-head slices of C)
  att   = softmax(mask(q k^T / sqrt(hd)))      mask: (q%256) >= (k%256)
  y     = (att @ v) @ Wp.T + bp

Sharding: 8 cores = 2 batches x 4 head-groups (3 heads each).
Each core computes its (batch, 3 heads) slab fully on-chip and returns a
partial pre-projection output out^T [768, 2560]; the host sums the 4
head-group partials per batch and adds the constant bias (bp + Wp @ bv).

v2 structure (per core):
  - qk projections (4 col-groups x 5 q-windows), v projection interleaved
    per window; bias adds on VectorE.
  - attention per unit (qt, h): 20 key chunks split into 5 uniform groups
    of 1536 score columns each (3 even chunks x 512, or mixed 512+256-wide
    odd blocks).  Mask written FIRST into PSUM via ident x maskm matmul
    with start=True (whole-bank has_written clear), scores accumulate on
    top, ScalarE exp (scale=1/8) -> bf16 pt, then V(+ones col) @ pt.
  - software pipelining: group g's AV matmuls are emitted after group
    g+1's score matmuls so the PE never sits behind the exp; the pipeline
    crosses unit boundaries.
  - softmax denominators: av[64] rows of the 3 heads of a q-window are
    copied to SBUF, bounced via DRAM into a [128,12] tile, ONE cheap
    reciprocal, bounced back, broadcast-DMA'd per head, one tensor_mul
    per head.  All bounce DMAs ride the idle GpSimd queue.
  - output projection per q-window deferred ~2 units so its inputs are
    always ready; results staged in SBUF and written with 2 wide DMAs.
"""

import numpy as np

B, T, C = 2, 2560, 768
H, HD = 12, 64
HPG = 3            # heads per group (core)
CW = HPG * HD      # 192
NKC = T // 128     # 20 key chunks of 128
NQT = T // 512     # 5 query tiles of 512
N_CORES = 8

_CACHE = {}


def _split_multi_waits(nc, maxw=1):
    """walrus in this container rejects >1 sync wait per instruction;
    split extra waits onto preceding NOPs on the same engine."""
    import concourse.mybir as mybir
    for f in nc.m.functions:
        for bb in f.blocks:
            newlist = []
            for ins in bb.instructions:
                si = ins.sync_info
                if si is not None and si.on_wait and len(si.on_wait) > maxw:
                    waits = list(si.on_wait)
                    chunks = [waits[i:i + maxw] for i in range(0, len(waits), maxw)]
                    for ch in chunks[:-1]:
                        newlist.append(mybir.InstNoOp(
                            name=f"WSPLIT-{nc.next_id()}",
                            engine=ins.engine,
                            sync_info=mybir.SyncInfo(on_wait=list(ch), on_update=[]),
                            text_hint="wait_split",
                        ))
                    ins.sync_info = mybir.SyncInfo(
                        on_wait=list(chunks[-1]), on_update=list(si.on_update))
                newlist.append(ins)
            bb.instructions = newlist
    return nc


def _unit_groups():
    """5 groups of 1536 score columns: [(kc, width, sc_offset), ...]."""
    gs = []
    for g in range(3):                      # E0,E1,E2: even chunks, 512 wide
        gs.append([(6 * g + 2 * i, 512, 512 * i) for i in range(3)])
    g3 = [(18, 512, 0)] + [(2 * i + 1, 256, 512 + 256 * i) for i in range(4)]
    gs.append(g3)                           # chunk 18 + odds 1,3,5,7
    gs.append([(2 * i + 1, 256, 256 * (i - 4)) for i in range(4, 10)])
    return gs                               # odds 9..19


def build_program():
    import concourse.bass as bass
    import concourse.mybir as mybir
    import concourse.tile as tile

    f32 = mybir.dt.float32
    bf16 = mybir.dt.bfloat16
    AF = mybir.ActivationFunctionType

    nc = bass.Bass()
    xT = nc.dram_tensor("xT", [C, T], bf16, kind="ExternalInput")
    wqk = nc.dram_tensor("wqk", [C, 384], bf16, kind="ExternalInput")
    bqk = nc.dram_tensor("bqk", [4, 128, 1], f32, kind="ExternalInput")
    wv = nc.dram_tensor("wv", [C, CW], bf16, kind="ExternalInput")
    wp = nc.dram_tensor("wp", [HPG, HD, C], bf16, kind="ExternalInput")
    identm = nc.dram_tensor("identm", [128, 640], f32, kind="ExternalInput")
    out = nc.dram_tensor("out", [C, T], f32, kind="ExternalOutput")
    denb = nc.dram_tensor("denb", [NQT, 1536], f32)
    rcpb = nc.dram_tensor("rcpb", [NQT, 1536], f32)

    with tile.TileContext(nc) as tc:
        with tc.tile_pool(name="persist", bufs=1) as persist, \
             tc.tile_pool(name="work", bufs=2) as work, \
             tc.tile_pool(name="psum", bufs=2, space="PSUM") as psum:

            # ---------------- input DMAs (critical path first) -------------
            wqk_sb = persist.tile([128, 6, 384], bf16)
            nc.sync.dma_start(out=wqk_sb, in_=bass.AP(
                tensor=wqk, offset=0,
                ap=[[384, 128], [128 * 384, 6], [1, 384]]))
            bqk_sb = persist.tile([128, 4], f32)
            nc.sync.dma_start(out=bqk_sb, in_=bass.AP(
                tensor=bqk, offset=0, ap=[[1, 128], [128, 4]]))
            xt_sb = persist.tile([128, 6, T], bf16)       # x^T, 6 chunks of C
            for c in range(6):
                nc.sync.dma_start(out=xt_sb[:, c, 0:512],
                                  in_=xT[c * 128:(c + 1) * 128, 0:512])
            # off-critical loads ride the gpsimd queue
            wv_sb = persist.tile([128, 6, CW], bf16)
            nc.gpsimd.dma_start(out=wv_sb, in_=bass.AP(
                tensor=wv, offset=0,
                ap=[[CW, 128], [128 * CW, 6], [1, CW]]))
            identm_f = work.tile([128, 640], f32, tag="im", bufs=1,
                                 name="identm_f")
            nc.gpsimd.dma_start(out=identm_f, in_=identm[:, :])
            wp_sb = persist.tile([64, HPG, C], bf16)
            for h in range(HPG):
                nc.gpsimd.dma_start(out=wp_sb[:, h, :], in_=wp[h])
            # rest of x
            for c in range(6):
                nc.sync.dma_start(out=xt_sb[:, c, 512:1024],
                                  in_=xT[c * 128:(c + 1) * 128, 512:1024])
            for c in range(6):
                nc.sync.dma_start(out=xt_sb[:, c, 1024:T],
                                  in_=xT[c * 128:(c + 1) * 128, 1024:T])

            ident_sb = persist.tile([128, 128], bf16)   # identity
            maskm4 = persist.tile([128, 512], bf16)     # [L|L|L|L]
            nc.vector.tensor_copy(ident_sb, identm_f[:, 0:128])
            nc.vector.tensor_copy(maskm4, identm_f[:, 128:640])

            # HAM warm-up: keep the PE busy during the initial DMA wait so
            # the clock gate is at 8/8 when the real matmuls arrive.
            wdum = persist.tile([128, 128], bf16)
            nc.vector.memset(wdum, 0.0)
            wps = psum.tile([128, 512], f32, tag="av", bufs=1, name="warm")
            for _ in range(40):
                nc.tensor.matmul(wps[:, 0:128], lhsT=wdum, rhs=wdum,
                                 start=True, stop=True, skip_group_check=True)

            # preload the Exp activation table off the critical path
            dummy = work.tile([128, 4], f32, tag="dmy", bufs=1, name="dummy")
            nc.scalar.activation(dummy, bqk_sb, AF.Exp, scale=0.0)

            # ---------------- q/k/v projections ----------------
            # qkT j0=[Qh0|Qh1] j1=[Kh0|Kh1] (128 parts); j2=Qh2 j3=Kh2 (64)
            qkT01q = persist.tile([128, T], bf16)
            qkT01k = persist.tile([128, T], bf16)
            qkT2q = persist.tile([64, T], bf16)
            qkT2k = persist.tile([64, T], bf16)
            jdefs = [(qkT01q, 0, 128, 0), (qkT01k, 128, 128, 1),
                     (qkT2q, 256, 64, 2), (qkT2k, 320, 64, 3)]

            # v natural layout + ones col: per head h cols [65h..65h+63]=V_h,
            # col 65h+64 = 1.0 (softmax denominator for free)
            v_sb = persist.tile([128, NKC, HPG * 65], bf16)
            v_r = v_sb.rearrange("p n (h c) -> p n h c", c=65)
            nc.vector.memset(v_r[:, :, :, 64], 1.0)

            for qt in range(NQT):
                for (dst, col0, m, j) in jdefs:
                    qk_ps = psum.tile([128, 512], f32, tag="sc", name="qk_ps")
                    for c in range(6):
                        nc.tensor.matmul(
                            qk_ps[:m, :],
                            lhsT=wqk_sb[:, c, col0:col0 + m],
                            rhs=xt_sb[:, c, qt * 512:(qt + 1) * 512],
                            start=(c == 0), stop=(c == 5))
                    nc.vector.tensor_scalar_add(
                        dst[:m, qt * 512:(qt + 1) * 512],
                        qk_ps[:m, :], bqk_sb[:m, j:j + 1])
                for tch in range(4 * qt, 4 * qt + 4):
                    v_ps = psum.tile([128, 512], f32, tag="sc", name="v_ps")
                    for c in range(6):
                        nc.tensor.matmul(
                            v_ps[:, :CW],
                            lhsT=xt_sb[:, c, tch * 128:(tch + 1) * 128],
                            rhs=wv_sb[:, c, :],
                            start=(c == 0), stop=(c == 5))
                    nc.vector.tensor_copy(
                        v_r[:, tch, :, 0:64],
                        v_ps[:, :CW].rearrange("p (h c) -> p h c", h=HPG))

            # ---------------- attention ----------------
            groups_def = _unit_groups()
            units = [(qt, h) for qt in range(NQT) for h in range(HPG)]
            avsb_q = {}
            bcs_q = {}
            pend = [None]  # (chunks, pt, av, h, last_of_unit, qt)
            fillers = []   # deferred normalize/proj work, one item per group

            def maybe_fill():
                if fillers:
                    fillers.pop(0)()

            def emit_masks(sc, G):
                for (kc, w, o) in G:
                    if w == 512:   # even chunk: L at t=0 half of each 256
                        mdst = sc[:, o:o + 512].rearrange(
                            "p (s t c) -> p s t c", s=2, t=2, c=128)[:, :, 0]
                        nc.tensor.matmul(mdst, lhsT=ident_sb,
                                         rhs=maskm4[:, 0:256],
                                         start=True, stop=False,
                                         skip_group_check=True)
                # odd chunks come in bank-aligned pairs: one 512-wide L|L|L|L
                odd_offsets = sorted(o for (kc, w, o) in G if w == 256)
                for o in odd_offsets[::2]:
                    nc.tensor.matmul(sc[:, o:o + 512], lhsT=ident_sb,
                                     rhs=maskm4,
                                     start=True, stop=False,
                                     skip_group_check=True)

            def emit_av(item):
                chunks, pt_t, av_t, h_t, last, qt_t = item
                av_odd = av_t.rearrange("p (s t c) -> p t s c",
                                        s=2, t=2, c=128)[:, 1]
                for (kc, w, o) in chunks:
                    if w == 512:
                        nc.tensor.matmul(
                            av_t[:65, :],
                            lhsT=v_sb[:, kc, 65 * h_t:65 * h_t + 65],
                            rhs=pt_t[:, o:o + 512],
                            start=(kc == 0), stop=(kc == NKC - 1),
                            skip_group_check=True)
                    else:
                        nc.tensor.matmul(
                            av_odd[:65],
                            lhsT=v_sb[:, kc, 65 * h_t:65 * h_t + 65],
                            rhs=pt_t[:, o:o + 256],
                            start=False, stop=(kc == NKC - 1),
                            skip_group_check=True)
                if last:
                    avsb = avsb_q[qt_t]
                    nc.vector.tensor_copy(avsb[:, h_t, :], av_t[0:65, :])
                    # bounce this unit's denominator row out right away
                    nc.sync.dma_start(
                        out=bass.AP(tensor=denb,
                                    offset=qt_t * 1536 + h_t * 512,
                                    ap=[[0, 1], [1, 512]]),
                        in_=avsb[64:65, h_t, :])
                    if h_t == HPG - 1:
                        emit_den_tail(qt_t)

            def emit_den_tail(qt_t):
                # restripe the 1536 denominators onto 128 partitions, one
                # cheap reciprocal, restripe back, broadcast per head
                d128 = work.tile([128, 12], f32, tag="d128", bufs=2,
                                 name="d128")
                nc.sync.dma_start(out=d128, in_=bass.AP(
                    tensor=denb, offset=qt_t * 1536, ap=[[1, 128], [128, 12]]))
                r128 = work.tile([128, 12], f32, tag="r128", bufs=2,
                                 name="r128")
                nc.vector.reciprocal(r128, d128)
                nc.sync.dma_start(
                    out=bass.AP(tensor=rcpb, offset=qt_t * 1536,
                                ap=[[1, 128], [128, 12]]),
                    in_=r128)
                bcs = []
                for h2 in range(HPG):
                    bc = work.tile([64, 512], f32, tag="bc", bufs=6,
                                   name="bc")
                    nc.sync.dma_start(out=bc, in_=bass.AP(
                        tensor=rcpb, offset=qt_t * 1536 + h2 * 512,
                        ap=[[0, 64], [1, 512]]))
                    bcs.append(bc)
                bcs_q[qt_t] = bcs

            def push_flush(qt_t):
                avsb = avsb_q.pop(qt_t)
                bcs = bcs_q.pop(qt_t)
                ynw = work.tile([64, HPG, 512], bf16, tag="ynw", bufs=2,
                                name="ynw")
                pjbuf = work.tile([128, 6, 512], f32, tag="pjb", bufs=2,
                                  name="pjbuf")

                def muls():
                    for h2 in range(HPG):
                        nc.vector.tensor_mul(ynw[:, h2, :],
                                             avsb[0:64, h2, :], bcs[h2])
                fillers.append(muls)

                def proj_m(m):
                    def go():
                        pj_ps = psum.tile([128, 512], f32, tag="pj", bufs=1,
                                          name="pj_ps")
                        for hh in range(HPG):
                            nc.tensor.matmul(
                                pj_ps,
                                lhsT=wp_sb[:, hh, m * 128:(m + 1) * 128],
                                rhs=ynw[:, hh, :],
                                start=(hh == 0), stop=(hh == 2))
                        nc.vector.tensor_copy(pjbuf[:, m, :], pj_ps)
                        if m == 2:
                            nc.gpsimd.dma_start(
                                out=bass.AP(
                                    tensor=out, offset=qt_t * 512,
                                    ap=[[T, 128], [128 * T, 3], [1, 512]]),
                                in_=pjbuf[:, 0:3, :])
                        elif m == 5:
                            nc.gpsimd.dma_start(
                                out=bass.AP(
                                    tensor=out,
                                    offset=3 * 128 * T + qt_t * 512,
                                    ap=[[T, 128], [128 * T, 3], [1, 512]]),
                                in_=pjbuf[:, 3:6, :])
                    return go
                for m in range(6):
                    fillers.append(proj_m(m))

            for (qt, h) in units:
                if h == 0:
                    avsb_q[qt] = work.tile([65, HPG, 512], f32, tag="avsb",
                                           bufs=3, name="avsb")
                    if qt >= 2:
                        push_flush(qt - 2)
                if h == 1 and qt == NQT - 1:
                    push_flush(NQT - 2)
                if h < 2:
                    qTh = qkT01q[64 * h:64 * (h + 1), :]
                    kTh = qkT01k[64 * h:64 * (h + 1), :]
                else:
                    qTh = qkT2q[0:64, :]
                    kTh = qkT2k[0:64, :]
                q_odd = qTh.rearrange("p (q s t c) -> p q t s c",
                                      q=NQT, s=2, t=2, c=128)
                qwin = qTh[:, qt * 512:(qt + 1) * 512]
                av_t = psum.tile([128, 512], f32, tag="av", bufs=1, name="av")

                for gi, G in enumerate(groups_def):
                    sc = psum.tile([128, 1536], f32, tag="sc", name="sc")
                    emit_masks(sc, G)
                    for (kc, w, o) in G:
                        if w == 512:
                            nc.tensor.matmul(
                                sc[:, o:o + 512],
                                lhsT=kTh[:, kc * 128:(kc + 1) * 128],
                                rhs=qwin, start=False, stop=True,
                                skip_group_check=True)
                        else:
                            nc.tensor.matmul(
                                sc[:, o:o + 256],
                                lhsT=kTh[:, kc * 128:(kc + 1) * 128],
                                rhs=q_odd[:, qt, 1], start=False, stop=True,
                                skip_group_check=True)
                    if pend[0] is not None:
                        emit_av(pend[0])
                    pt_t = work.tile([128, 1536], bf16, tag="pt", bufs=3,
                                     name="pt")
                    nc.scalar.activation(pt_t, sc, AF.Exp, scale=0.125)
                    pend[0] = (G, pt_t, av_t, h,
                               gi == len(groups_def) - 1, qt)
                    maybe_fill()

            emit_av(pend[0])
            while fillers:
                fillers.pop(0)()
            push_flush(NQT - 1)
            while fillers:
                fillers.pop(0)()

    _split_multi_waits(nc)
    return nc


def get_program():
    if "nc" not in _CACHE:
        _CACHE["nc"] = build_program()
    return _CACHE["nc"]


def make_in_maps(x, Wk, bk, Wq, bq, Wv, bv, Wp, bp):
    x = np.asarray(x, dtype=np.float32)
    in_maps = []
    for core in range(N_CORES):
        b, g = divmod(core, 4)
        h0 = g * HPG
        r = slice(h0 * HD, (h0 + HPG) * HD)     # 192 head dims
        xt = np.ascontiguousarray(x[b].T)
        wq_g = np.asarray(Wq)[r]                 # [192, 768]
        wk_g = np.asarray(Wk)[r]
        # wqk cols: [Qh0|Qh1(128) | Kh0|Kh1(128) | Qh2(64) | Kh2(64)]
        wqk = np.concatenate(
            [wq_g[:128].T, wk_g[:128].T, wq_g[128:].T, wk_g[128:].T],
            axis=1).astype(np.float32)
        bq_g = np.asarray(bq)[r].astype(np.float32)
        bk_g = np.asarray(bk)[r].astype(np.float32)
        bqk = np.zeros((4, 128, 1), np.float32)
        bqk[0, :, 0] = bq_g[:128]
        bqk[1, :, 0] = bk_g[:128]
        bqk[2, :64, 0] = bq_g[128:]
        bqk[3, :64, 0] = bk_g[128:]
        wv_g = np.ascontiguousarray(np.asarray(Wv)[r].T).astype(np.float32)
        wp_g = np.asarray(Wp)[:, r]              # [768, 192]
        wp_t = np.ascontiguousarray(
            wp_g.T.reshape(HPG, HD, C)).astype(np.float32)
        ident = np.eye(128, dtype=np.float32)
        L = np.where(np.arange(128)[None, :] < np.arange(128)[:, None],
                     np.float32(-1e9), np.float32(0.0))
        identm = np.concatenate([ident, np.tile(L, (1, 4))],
                                axis=1).astype(np.float32)
        import ml_dtypes
        b16 = ml_dtypes.bfloat16
        in_maps.append({
            "identm": identm,
            "xT": np.ascontiguousarray(xt).astype(b16),
            "wqk": np.ascontiguousarray(wqk).astype(b16),
            "bqk": bqk,
            "wv": wv_g.astype(b16),
            "wp": wp_t.astype(b16),
        })
    return in_maps


def kernel(x, Wk, bk, Wq, bq, Wv, bv, Wp, bp):
    from concourse.bass_utils import run_bass_kernel_spmd
    nc = get_program()
    in_maps = make_in_maps(x, Wk, bk, Wq, bq, Wv, bv, Wp, bp)
    res = run_bass_kernel_spmd(nc, in_maps, list(range(N_CORES)))
    Wp_np = np.asarray(Wp, dtype=np.float32)
    const = (np.asarray(bp, dtype=np.float32)
             + Wp_np @ np.asarray(bv, dtype=np.float32))   # [768]
    out = np.empty((B, T, C), dtype=np.float32)
    for b in range(B):
        acc = res.results[b * 4 + 0]["out"].astype(np.float32).copy()
        for g in range(1, 4):
            acc += res.results[b * 4 + g]["out"]
        out[b] = acc.T + const[None, :]
    return out
